# revision 6
# baseline (speedup 1.0000x reference)
"""EntNet forward kernel for 8 Trainium2 NeuronCores (Bass/Tile).

Math note: in the reference, the gated memory is
    mem = memory_nodes * (1 + sigmoid(...))
followed by per-column L2 normalization.  Since (1 + sigmoid(x)) > 0 is a
per-column positive scalar, it cancels exactly in the normalization, so the
gate g — and with it s_in, F_i, input, keys, U, V, W, a_mem (cand is dead in
the source already) — does not affect the output.  Live computation:

    s_q = F_q @ query[0]                         # [D]
    mn  = memory_nodes / max(||col||_2, 1e-12)   # [D, M] column-normalized
    p   = softmax(s_q^T @ mn)                    # [1, M]
    u   = mn @ p^T                               # [D]
    y   = R @ prelu(s_q + H @ u, a_out)          # [D, 1]

Sharding: D is row-sharded over 8 cores (rows_c = 512c:512c+512).  Each core
streams its shards of F_q^T, mem, mem^T (row shards), H^T (H column shard),
R^T (R row shard) — ~36 MiB/core vs 608 MiB of live bytes single-core.  Two
8/16 KB AllReduces chain the serial tail:
  AR1: [r | column-sum-of-squares] partials ([128, KM*2])
  AR2: z = s_q + H @ u partials     ([128, KZ])
Vectors are kept partition-major throughout: v[128i + p] <-> tile[p, i].
"""

import sys
import numpy as np

for _p in ("/root/.axon_site/_ro/trn_rl_repo", "/opt/trn_rl_repo"):
    if _p not in sys.path:
        sys.path.append(_p)

D, M, L = 4096, 1024, 8192
N_CORES = 8

_CACHE = {}


def _build_module(n_cores, d, m, l):  # noqa: E741
    import concourse.bacc as bacc
    import concourse.tile as tile
    import concourse.mybir as mybir
    import concourse.bass_isa as bass_isa

    f32 = mybir.dt.float32
    DL = d // n_cores      # local rows of D
    KD = DL // 128         # local d chunks
    KM = m // 128          # m chunks
    KZ = d // 128          # global d chunks
    KL = l // 128          # l chunks
    AF = mybir.ActivationFunctionType

    # F_q^T streamed in chunks of FQ_N l-tiles (<=16 -> 4 MiB at full size)
    FQ_N = min(16, KL)
    assert KL % FQ_N == 0
    # R^T streamed in chunks of RT_N j-tiles (4 -> 1 MiB at full size)
    RT_N = min(4, KZ)
    assert KZ % RT_N == 0

    nc = bacc.Bacc("TRN2", target_bir_lowering=False, debug=False,
                   enable_asserts=False, num_devices=n_cores)

    fqT_in = nc.dram_tensor("fqT", [l, DL], f32, kind="ExternalInput")
    q2d_in = nc.dram_tensor("q2d", [128, KL], f32, kind="ExternalInput")
    memd_in = nc.dram_tensor("memd", [DL, m], f32, kind="ExternalInput")
    memT_in = nc.dram_tensor("memT", [m, DL], f32, kind="ExternalInput")
    hT_in = nc.dram_tensor("hT", [DL, d], f32, kind="ExternalInput")
    rT_in = nc.dram_tensor("rT", [d, DL], f32, kind="ExternalInput")
    ab_in = nc.dram_tensor("ab", [128, 1], f32, kind="ExternalInput")
    mask_in = nc.dram_tensor("mask", [128, KZ], f32, kind="ExternalInput")
    y_out = nc.dram_tensor("y", [1, DL], f32, kind="ExternalOutput")

    with tile.TileContext(nc) as tc:
        with (
            tc.tile_pool(name="consts", bufs=1) as consts,
            tc.tile_pool(name="fq", bufs=2) as fqp,
            tc.tile_pool(name="mem", bufs=1) as memp,
            tc.tile_pool(name="big", bufs=1) as bigp,
            tc.tile_pool(name="rt", bufs=3) as rtp,
            tc.tile_pool(name="sm", bufs=1) as smp,
            tc.tile_pool(name="sq_scratch", bufs=2) as scrp,
            tc.tile_pool(name="ps_sq", bufs=1, space="PSUM") as ps_sq,
            tc.tile_pool(name="ps_r", bufs=1, space="PSUM") as ps_r,
            tc.tile_pool(name="ps_u", bufs=1, space="PSUM") as ps_u,
            tc.tile_pool(name="ps_z", bufs=1, space="PSUM") as ps_z,
            tc.tile_pool(name="ps_y", bufs=1, space="PSUM") as ps_y,
            tc.tile_pool(name="dram", bufs=1, space="DRAM") as dram,
        ):
            # ---- small constants ----
            q2d = consts.tile([128, KL], f32)
            nc.sync.dma_start(q2d[:], q2d_in[:])
            ab = consts.tile([128, 1], f32)
            nc.sync.dma_start(ab[:], ab_in[:])
            mask = consts.tile([128, KZ], f32)
            nc.sync.dma_start(mask[:], mask_in[:])

            # ---- phase 1: s_q = F_q @ query, streaming F_q^T ----
            psq = ps_sq.tile([128, KD], f32)
            fqT_r = fqT_in[:].rearrange("(n p) e -> p n e", p=128)
            for i in range(KL // FQ_N):
                fq_t = fqp.tile([128, FQ_N, DL], f32)
                nc.sync.dma_start(fq_t[:], fqT_r[:, FQ_N * i:FQ_N * (i + 1), :])
                for j in range(FQ_N):
                    n = FQ_N * i + j
                    for dt in range(KD):
                        nc.tensor.matmul(
                            psq[:, dt:dt + 1],
                            fq_t[:, j, 128 * dt:128 * (dt + 1)],
                            q2d[:, n:n + 1],
                            start=(n == 0 and dt == 0),
                            stop=(n == KL - 1 and dt == KD - 1),
                        )
            s_q = smp.tile([128, KD], f32)
            nc.vector.tensor_copy(s_q[:], psq[:])

            # ---- mem shards in, column sum-of-squares via ACT square ----
            memd = memp.tile([128, KD, m], f32)
            nc.sync.dma_start(memd[:], memd_in[:].rearrange("(b p) m -> p b m", p=128))
            memT = memp.tile([128, KM, DL], f32)
            nc.sync.dma_start(memT[:], memT_in[:].rearrange("(b p) e -> p b e", p=128))

            # H^T resident (needed right after softmax)
            hT = bigp.tile([128, KD, d], f32)
            nc.sync.dma_start(hT[:], hT_in[:].rearrange("(b p) e -> p b e", p=128))

            ar1 = smp.tile([128, 2 * KM], f32)
            for b in range(KM):
                scratch = scrp.tile([128, DL], f32)
                nc.scalar.activation(
                    scratch[:], memT[:, b, :], AF.Square,
                    accum_out=ar1[:, KM + b:KM + b + 1],
                )

            # ---- r = mem^T @ s_q (local-d partial) ----
            pr = ps_r.tile([128, KM], f32)
            for mt in range(KM):
                for kc in range(KD):
                    nc.tensor.matmul(
                        pr[:, mt:mt + 1],
                        memd[:, kc, 128 * mt:128 * (mt + 1)],
                        s_q[:, kc:kc + 1],
                        start=(mt == 0 and kc == 0),
                        stop=(mt == KM - 1 and kc == KD - 1),
                    )
            nc.vector.tensor_copy(ar1[:, 0:KM], pr[:])

            # ---- AllReduce 1: [r | ss] ----
            car1_i = dram.tile([128, 2 * KM], f32)
            car1_o = dram.tile([128, 2 * KM], f32)
            nc.gpsimd.dma_start(car1_i[:], ar1[:])
            nc.gpsimd.collective_compute(
                "AllReduce", mybir.AluOpType.add,
                replica_groups=[list(range(n_cores))],
                ins=[car1_i[:].opt()], outs=[car1_o[:].opt()],
            )
            f1 = smp.tile([128, 2 * KM], f32)
            nc.gpsimd.dma_start(f1[:], car1_o[:])

            # ---- softmax (partition-major), fold 1/denom into p ----
            dn = smp.tile([128, KM], f32)
            nc.scalar.activation(dn[:], f1[:, KM:2 * KM], AF.Sqrt)
            dnm = smp.tile([128, KM], f32)
            nc.vector.tensor_scalar_max(dnm[:], dn[:], 1e-12)
            rdn = smp.tile([128, KM], f32)
            nc.vector.reciprocal(rdn[:], dnm[:])
            t = smp.tile([128, KM], f32)
            nc.vector.tensor_mul(t[:], f1[:, 0:KM], rdn[:])
            tm = smp.tile([128, 1], f32)
            nc.vector.tensor_reduce(tm[:], t[:], mybir.AxisListType.X,
                                    mybir.AluOpType.max)
            tmb = smp.tile([128, 1], f32)
            nc.gpsimd.partition_all_reduce(tmb[:], tm[:], 128,
                                           bass_isa.ReduceOp.max)
            negmx = smp.tile([128, 1], f32)
            nc.vector.tensor_scalar_mul(negmx[:], tmb[:], -1.0)
            e = smp.tile([128, KM], f32)
            esum = smp.tile([128, 1], f32)
            nc.scalar.activation(e[:], t[:], AF.Exp, bias=negmx[:],
                                 accum_out=esum[:])
            esb = smp.tile([128, 1], f32)
            nc.gpsimd.partition_all_reduce(esb[:], esum[:], 128,
                                           bass_isa.ReduceOp.add)
            sd = smp.tile([128, KM], f32)
            nc.vector.tensor_scalar_mul(sd[:], dnm[:], esb[:])
            rsd = smp.tile([128, KM], f32)
            nc.vector.reciprocal(rsd[:], sd[:])
            pt = smp.tile([128, KM], f32)
            nc.vector.tensor_mul(pt[:], e[:], rsd[:])

            # ---- u = mem @ (p/denom), local rows ----
            pu = ps_u.tile([128, KD], f32)
            for dt in range(KD):
                for kc in range(KM):
                    nc.tensor.matmul(
                        pu[:, dt:dt + 1],
                        memT[:, kc, 128 * dt:128 * (dt + 1)],
                        pt[:, kc:kc + 1],
                        start=(dt == 0 and kc == 0),
                        stop=(dt == KD - 1 and kc == KM - 1),
                    )
            u_sb = smp.tile([128, KD], f32)
            nc.vector.tensor_copy(u_sb[:], pu[:])

            # ---- z partial = H[:, cols_c] @ u_c  (full-D, partition-major) ----
            pz = ps_z.tile([128, KZ], f32)
            for kc in range(KD):
                for dt in range(KZ):
                    nc.tensor.matmul(
                        pz[:, dt:dt + 1],
                        hT[:, kc, 128 * dt:128 * (dt + 1)],
                        u_sb[:, kc:kc + 1],
                        start=(kc == 0 and dt == 0),
                        stop=(kc == KD - 1 and dt == KZ - 1),
                    )

            # place this core's s_q shard via the per-core mask, add to z
            sqm = smp.tile([128, KZ], f32)
            nc.vector.tensor_tensor(
                sqm[:].rearrange("p (r k) -> p r k", k=KD),
                s_q[:].unsqueeze(1).broadcast_to([128, KZ // KD, KD]),
                mask[:].rearrange("p (r k) -> p r k", k=KD),
                mybir.AluOpType.mult,
            )
            ar2 = smp.tile([128, KZ], f32)
            nc.vector.tensor_add(ar2[:], sqm[:], pz[:])

            # ---- AllReduce 2: z ----
            car2_i = dram.tile([128, KZ], f32)
            car2_o = dram.tile([128, KZ], f32)
            nc.gpsimd.dma_start(car2_i[:], ar2[:])
            nc.gpsimd.collective_compute(
                "AllReduce", mybir.AluOpType.add,
                replica_groups=[list(range(n_cores))],
                ins=[car2_i[:].opt()], outs=[car2_o[:].opt()],
            )
            zf = smp.tile([128, KZ], f32)
            nc.gpsimd.dma_start(zf[:], car2_o[:])

            # ---- prelu(z) = relu(z) + a * (z - relu(z)) ----
            pos = smp.tile([128, KZ], f32)
            nc.scalar.activation(pos[:], zf[:], AF.Relu)
            neg = smp.tile([128, KZ], f32)
            nc.vector.tensor_sub(neg[:], zf[:], pos[:])
            negs = smp.tile([128, KZ], f32)
            nc.vector.tensor_scalar_mul(negs[:], neg[:], ab[:])
            pzz = smp.tile([128, KZ], f32)
            nc.vector.tensor_add(pzz[:], pos[:], negs[:])

            # ---- y = R[rows_c] @ prelu(z), streaming R^T ----
            py = ps_y.tile([1, DL], f32)
            rT_r = rT_in[:].rearrange("(n p) e -> p n e", p=128)
            for i in range(KZ // RT_N):
                rt_t = rtp.tile([128, RT_N, DL], f32)
                nc.sync.dma_start(rt_t[:], rT_r[:, RT_N * i:RT_N * (i + 1), :])
                for j in range(RT_N):
                    kc = RT_N * i + j
                    nc.tensor.matmul(
                        py[:, :],
                        pzz[:, kc:kc + 1],
                        rt_t[:, j, :],
                        start=(kc == 0), stop=(kc == KZ - 1),
                    )
            y_sb = smp.tile([1, DL], f32)
            nc.vector.tensor_copy(y_sb[:], py[:])
            nc.gpsimd.dma_start(y_out[:], y_sb[:])

    nc.compile()
    return nc


def _get_module(n_cores=N_CORES, d=D, m=M, l=L):  # noqa: E741
    key = (n_cores, d, m, l)
    if key not in _CACHE:
        _CACHE[key] = _build_module(n_cores, d, m, l)
    return _CACHE[key]


def _make_in_maps(n_cores, d, m, l, F_q, query, memory_nodes, H, R, a_out):  # noqa: E741
    f32 = np.float32
    DL = d // n_cores
    KZ = d // 128
    KD = DL // 128
    KL = l // 128
    q2d = np.ascontiguousarray(query.reshape(KL, 128).T).astype(f32, copy=False)
    in_maps = []
    for c in range(n_cores):
        rows = slice(DL * c, DL * (c + 1))
        mask = np.zeros((128, KZ), f32)
        mask[:, KD * c:KD * (c + 1)] = 1.0
        in_maps.append({
            "fqT": np.ascontiguousarray(F_q[rows].T),
            "q2d": q2d,
            "memd": np.ascontiguousarray(memory_nodes[rows]),
            "memT": np.ascontiguousarray(memory_nodes[rows].T),
            "hT": np.ascontiguousarray(H[:, rows].T),
            "rT": np.ascontiguousarray(R[rows].T),
            "ab": np.full((128, 1), a_out, f32),
            "mask": mask,
        })
    return in_maps


class _PjrtRunner:
    """Cached jit(shard_map(bass_exec)) so repeat kernel() calls skip
    retracing/recompiling (bass_utils.run_bass_kernel_spmd rebuilds the jit
    closure every call)."""

    def __init__(self, nc, n_cores):
        import jax
        from jax.sharding import Mesh, PartitionSpec
        from jax.experimental.shard_map import shard_map
        from concourse import bass2jax
        import concourse.mybir as mybir

        bass2jax.install_neuronx_cc_hook()
        self.n_cores = n_cores
        part_name = (nc.partition_id_tensor.name
                     if nc.partition_id_tensor else None)
        in_names, out_names, out_avals = [], [], []
        for alloc in nc.m.functions[0].allocations:
            if not isinstance(alloc, mybir.MemoryLocationSet):
                continue
            name = alloc.memorylocations[0].name
            if alloc.kind == "ExternalInput":
                if name != part_name:
                    in_names.append(name)
            elif alloc.kind == "ExternalOutput":
                out_names.append(name)
                out_avals.append(jax.core.ShapedArray(
                    tuple(alloc.tensor_shape), mybir.dt.np(alloc.dtype)))
        self.in_names, self.out_names, self.out_avals = in_names, out_names, out_avals
        n_params = len(in_names)
        self.zero_outs = [np.zeros(a.shape, a.dtype) for a in out_avals]
        all_in_names = tuple(in_names + out_names)
        if part_name is not None:
            all_in_names = all_in_names + (part_name,)

        def _body(*args):
            operands = list(args)
            if part_name is not None:
                operands.append(bass2jax.partition_id_tensor())
            outs = bass2jax._bass_exec_p.bind(
                *operands,
                out_avals=tuple(out_avals),
                in_names=all_in_names,
                out_names=tuple(out_names),
                lowering_input_output_aliases=(),
                sim_require_finite=True,
                sim_require_nnan=True,
                nc=nc,
            )
            return tuple(outs)

        devices = jax.devices()[:n_cores]
        mesh = Mesh(np.asarray(devices), ("core",))
        n_out = len(out_names)
        self._fn = jax.jit(
            shard_map(
                _body, mesh=mesh,
                in_specs=(PartitionSpec("core"),) * (n_params + n_out),
                out_specs=(PartitionSpec("core"),) * n_out,
                check_rep=False,
            ),
            keep_unused=True,
        )

    def __call__(self, in_maps):
        n = self.n_cores
        concat_in = [
            np.concatenate([in_maps[c][name] for c in range(n)], axis=0)
            for name in self.in_names
        ]
        concat_zeros = [
            np.zeros((n * z.shape[0], *z.shape[1:]), z.dtype)
            for z in self.zero_outs
        ]
        out_arrs = self._fn(*concat_in, *concat_zeros)
        return [
            {name: np.asarray(out_arrs[i]).reshape(n, *self.out_avals[i].shape)[c]
             for i, name in enumerate(self.out_names)}
            for c in range(n)
        ]


_RUNNER = {}


def _get_runner():
    if "r" not in _RUNNER:
        _RUNNER["r"] = _PjrtRunner(_get_module(), N_CORES)
    return _RUNNER["r"]


def kernel(**inputs):
    f32 = np.float32
    F_q = np.asarray(inputs["F_q"], f32)
    query = np.asarray(inputs["query"], f32).reshape(-1)
    memory_nodes = np.asarray(inputs["memory_nodes"], f32)
    H = np.asarray(inputs["H"], f32)
    R = np.asarray(inputs["R"], f32)
    a_out = float(np.asarray(inputs["a_out"]).reshape(-1)[0])

    in_maps = _make_in_maps(N_CORES, D, M, L, F_q, query, memory_nodes,
                            H, R, a_out)
    results = _get_runner()(in_maps)
    y = np.concatenate([results[c]["y"].reshape(-1) for c in range(N_CORES)])
    return y.reshape(D, 1).astype(f32)


# revision 7
# speedup vs baseline: 1.3279x; 1.3279x over previous
"""EntNet forward kernel for 8 Trainium2 NeuronCores (Bass/Tile).

Math note: in the reference, the gated memory is
    mem = memory_nodes * (1 + sigmoid(...))
followed by per-column L2 normalization.  Since (1 + sigmoid(x)) > 0 is a
per-column positive scalar, it cancels exactly in the normalization, so the
gate g — and with it s_in, F_i, input, keys, U, V, W, a_mem (cand is dead in
the source already) — does not affect the output.  Live computation:

    s_q = F_q @ query[0]                         # [D]
    mn  = memory_nodes / max(||col||_2, 1e-12)   # [D, M] column-normalized
    p   = softmax(s_q^T @ mn)                    # [1, M]
    u   = mn @ p^T                               # [D]
    y   = R @ prelu(s_q + H @ u, a_out)          # [D, 1]

Sharding: D is row-sharded over 8 cores (rows_c = 512c:512c+512).  Each core
streams its shards of F_q^T, mem, mem^T (row shards), H^T (H column shard),
R^T (R row shard) — ~36 MiB/core vs 608 MiB of live bytes single-core.
Collectives: an early AllReduce of the column sum-of-squares (also absorbs
the cold-start cost of the collective machinery), a second 4 KB AllReduce of
the r = mem^T s_q partials, and a 16 KB AllReduce of z = s_q + H@u partials.

Precision: f32 PE matvecs measure ~430 ns per 128x128 weight tile on trn2,
while bf16 weight-stationary tiles pipeline at ~27 ns.  All matrices are
shipped as bf16 (hi, lo) splits — X = hi + lo with hi = bf16(X),
lo = bf16(X - hi) — and each matvec computes hi@vhi + hi@vlo + lo@vhi in
f32 PSUM (the dropped lo@lo term is O(2^-18) relative).  Same DMA bytes as
f32, ~1e-5 relative output error, >5x PE speedup.

Vectors are partition-major throughout: v[128i + p] <-> tile[p, i].
"""

import sys
import numpy as np

for _p in ("/root/.axon_site/_ro/trn_rl_repo", "/opt/trn_rl_repo"):
    if _p not in sys.path:
        sys.path.append(_p)

D, M, L = 4096, 1024, 8192
N_CORES = 8

_CACHE = {}


def _build_module(n_cores, d, m, l):  # noqa: E741
    import concourse.bacc as bacc
    import concourse.tile as tile
    import concourse.mybir as mybir
    import concourse.bass_isa as bass_isa

    f32 = mybir.dt.float32
    bf16 = mybir.dt.bfloat16
    DL = d // n_cores      # local rows of D
    KD = DL // 128         # local d chunks
    KM = m // 128          # m chunks
    KZ = d // 128          # global d chunks
    KL = l // 128          # l chunks
    AF = mybir.ActivationFunctionType
    ADD = mybir.AluOpType.add
    rg = [list(range(n_cores))]

    # F_q^T hi|lo streamed in chunks of FQ_N l-tiles (8 -> 1 MiB at full size)
    FQ_N = min(8, KL)
    assert (2 * KL) % FQ_N == 0
    RT_N = min(8, KZ)
    assert (2 * KZ) % RT_N == 0

    nc = bacc.Bacc("TRN2", target_bir_lowering=False, debug=False,
                   enable_asserts=False, num_devices=n_cores)

    # hi|lo packed along the leading axis: rows [0:n] = hi, [n:2n] = lo
    fqT_in = nc.dram_tensor("fqT", [2 * l, DL], bf16, kind="ExternalInput")
    q2d_in = nc.dram_tensor("q2d", [128, 2 * KL], bf16, kind="ExternalInput")
    memd_in = nc.dram_tensor("memd", [2 * DL, m], bf16, kind="ExternalInput")
    memT_in = nc.dram_tensor("memT", [2 * m, DL], bf16, kind="ExternalInput")
    hT_in = nc.dram_tensor("hT", [2 * DL, d], bf16, kind="ExternalInput")
    rT_in = nc.dram_tensor("rT", [2 * d, DL], bf16, kind="ExternalInput")
    ab_in = nc.dram_tensor("ab", [128, 1], f32, kind="ExternalInput")
    mask_in = nc.dram_tensor("mask", [128, KZ], f32, kind="ExternalInput")
    y_out = nc.dram_tensor("y", [1, DL], f32, kind="ExternalOutput")

    with tile.TileContext(nc) as tc:
        with (
            tc.tile_pool(name="consts", bufs=1) as consts,
            tc.tile_pool(name="fq", bufs=3) as fqp,
            tc.tile_pool(name="mem", bufs=1) as memp,
            tc.tile_pool(name="big", bufs=1) as bigp,
            tc.tile_pool(name="rt", bufs=3) as rtp,
            tc.tile_pool(name="sm", bufs=1) as smp,
            tc.tile_pool(name="scr", bufs=2) as scrp,
            tc.tile_pool(name="ps_sq", bufs=1, space="PSUM") as ps_sq,
            tc.tile_pool(name="ps_r", bufs=1, space="PSUM") as ps_r,
            tc.tile_pool(name="ps_u", bufs=1, space="PSUM") as ps_u,
            tc.tile_pool(name="ps_z", bufs=1, space="PSUM") as ps_z,
            tc.tile_pool(name="ps_y", bufs=1, space="PSUM") as ps_y,
            tc.tile_pool(name="dram", bufs=1, space="DRAM") as dram,
        ):
            # ---- ACT table warmup (Sqrt/Exp/Relu load ~1.3us each lazily) ----
            warm = consts.tile([1, 1], f32)
            nc.gpsimd.memset(warm[:], 1.0)
            w2 = consts.tile([1, 1], f32)
            nc.scalar.activation(w2[:], warm[:], AF.Square)
            nc.scalar.activation(w2[:], warm[:], AF.Sqrt)
            nc.scalar.activation(w2[:], warm[:], AF.Exp)
            nc.scalar.activation(w2[:], warm[:], AF.Relu)

            # ---- small constants ----
            q2d = consts.tile([128, 2 * KL], bf16)
            nc.sync.dma_start(q2d[:], q2d_in[:])
            ab = consts.tile([128, 1], f32)
            nc.sync.dma_start(ab[:], ab_in[:])
            mask = consts.tile([128, KZ], f32)
            nc.sync.dma_start(mask[:], mask_in[:])

            def split_hl(src_f32, k, name):
                """f32 [128, k] -> bf16 [128, 2k] = [hi | lo]."""
                hl = smp.tile([128, 2 * k], bf16, name=name)
                nc.vector.tensor_copy(hl[:, 0:k], src_f32)
                hi_f = smp.tile([128, k], f32, name=name + "_hf")
                nc.vector.tensor_copy(hi_f[:], hl[:, 0:k])
                lo_f = smp.tile([128, k], f32, name=name + "_lf")
                nc.vector.tensor_sub(lo_f[:], src_f32, hi_f[:])
                nc.vector.tensor_copy(hl[:, k:2 * k], lo_f[:])
                return hl

            # ---- mem shards; column sum-of-squares -> early AllReduce ----
            memd = memp.tile([128, 2 * KD, m], bf16)
            nc.sync.dma_start(memd[:], memd_in[:].rearrange("(b p) m -> p b m", p=128))
            memT = memp.tile([128, 2 * KM, DL], bf16)
            nc.sync.dma_start(memT[:], memT_in[:].rearrange("(b p) e -> p b e", p=128))

            ss = smp.tile([128, KM], f32)
            for b in range(KM):
                full = scrp.tile([128, DL], f32, name="ssfull")
                nc.vector.tensor_add(full[:], memT[:, b, :], memT[:, KM + b, :])
                scratch = scrp.tile([128, DL], f32, name="sssq")
                nc.scalar.activation(scratch[:], full[:], AF.Square,
                                     accum_out=ss[:, b:b + 1])
            css_i = dram.tile([128, KM], f32)
            css_o = dram.tile([128, KM], f32)
            nc.gpsimd.dma_start(css_i[:], ss[:])
            nc.gpsimd.collective_compute(
                "AllReduce", ADD, replica_groups=rg,
                ins=[css_i[:].opt()], outs=[css_o[:].opt()])
            ssf = smp.tile([128, KM], f32)
            nc.gpsimd.dma_start(ssf[:], css_o[:])

            # ---- phase 1: s_q = F_q @ query, streaming F_q^T (hi then lo) ----
            psq = ps_sq.tile([128, KD], f32)
            fqT_r = fqT_in[:].rearrange("(n p) e -> p n e", p=128)
            n_mm = 0
            N_MM_TOT = 2 * KL * KD + KL * KD
            for i in range((2 * KL) // FQ_N):
                fq_t = fqp.tile([128, FQ_N, DL], bf16)
                nc.sync.dma_start(fq_t[:], fqT_r[:, FQ_N * i:FQ_N * (i + 1), :])
                for j in range(FQ_N):
                    n = FQ_N * i + j
                    is_hi = n < KL
                    nl = n if is_hi else n - KL
                    # hi tile: hi@qhi + hi@qlo ; lo tile: lo@qhi
                    rhs_cols = ([nl, KL + nl] if is_hi else [nl])
                    for dt in range(KD):
                        for col in rhs_cols:
                            nc.tensor.matmul(
                                psq[:, dt:dt + 1],
                                fq_t[:, j, 128 * dt:128 * (dt + 1)],
                                q2d[:, col:col + 1],
                                start=(n_mm == 0), stop=(n_mm == N_MM_TOT - 1),
                            )
                            n_mm += 1
            s_q = smp.tile([128, KD], f32)
            nc.vector.tensor_copy(s_q[:], psq[:])
            sq_hl = split_hl(s_q[:], KD, "sq_hl")

            # H^T resident (needed right after softmax)
            hT = bigp.tile([128, 2 * KD, d], bf16)
            nc.sync.dma_start(hT[:], hT_in[:].rearrange("(b p) e -> p b e", p=128))

            # ---- r = mem^T @ s_q (local-d partial) ----
            pr = ps_r.tile([128, KM], f32)
            n_mm = 0
            N_MM_TOT = 3 * KM * KD
            for mt in range(KM):
                for kc in range(KD):
                    for (bw, col) in ((kc, 0), (kc, KD), (KD + kc, 0)):
                        nc.tensor.matmul(
                            pr[:, mt:mt + 1],
                            memd[:, bw, 128 * mt:128 * (mt + 1)],
                            sq_hl[:, col + kc:col + kc + 1],
                            start=(n_mm == 0), stop=(n_mm == N_MM_TOT - 1),
                        )
                        n_mm += 1
            r_sb = smp.tile([128, KM], f32)
            nc.vector.tensor_copy(r_sb[:], pr[:])

            # ---- AllReduce: r ----
            cr_i = dram.tile([128, KM], f32)
            cr_o = dram.tile([128, KM], f32)
            nc.gpsimd.dma_start(cr_i[:], r_sb[:])
            nc.gpsimd.collective_compute(
                "AllReduce", ADD, replica_groups=rg,
                ins=[cr_i[:].opt()], outs=[cr_o[:].opt()])
            rf = smp.tile([128, KM], f32)
            nc.gpsimd.dma_start(rf[:], cr_o[:])

            # ---- softmax (partition-major), fold 1/denom into p ----
            dn = smp.tile([128, KM], f32)
            nc.scalar.activation(dn[:], ssf[:], AF.Sqrt)
            dnm = smp.tile([128, KM], f32)
            nc.vector.tensor_scalar_max(dnm[:], dn[:], 1e-12)
            rdn = smp.tile([128, KM], f32)
            nc.vector.reciprocal(rdn[:], dnm[:])
            t = smp.tile([128, KM], f32)
            nc.vector.tensor_mul(t[:], rf[:], rdn[:])
            tm = smp.tile([128, 1], f32)
            nc.vector.tensor_reduce(tm[:], t[:], mybir.AxisListType.X,
                                    mybir.AluOpType.max)
            tmb = smp.tile([128, 1], f32)
            nc.gpsimd.partition_all_reduce(tmb[:], tm[:], 128,
                                           bass_isa.ReduceOp.max)
            negmx = smp.tile([128, 1], f32)
            nc.vector.tensor_scalar_mul(negmx[:], tmb[:], -1.0)
            e = smp.tile([128, KM], f32)
            esum = smp.tile([128, 1], f32)
            nc.scalar.activation(e[:], t[:], AF.Exp, bias=negmx[:],
                                 accum_out=esum[:])
            esb = smp.tile([128, 1], f32)
            nc.gpsimd.partition_all_reduce(esb[:], esum[:], 128,
                                           bass_isa.ReduceOp.add)
            sd = smp.tile([128, KM], f32)
            nc.vector.tensor_scalar_mul(sd[:], dnm[:], esb[:])
            rsd = smp.tile([128, KM], f32)
            nc.vector.reciprocal(rsd[:], sd[:])
            pt = smp.tile([128, KM], f32)
            nc.vector.tensor_mul(pt[:], e[:], rsd[:])
            pt_hl = split_hl(pt[:], KM, "pt_hl")

            # ---- u = mem @ (p/denom), local rows ----
            pu = ps_u.tile([128, KD], f32)
            n_mm = 0
            N_MM_TOT = 3 * KD * KM
            for dt in range(KD):
                for kc in range(KM):
                    for (bw, col) in ((kc, 0), (kc, KM), (KM + kc, 0)):
                        nc.tensor.matmul(
                            pu[:, dt:dt + 1],
                            memT[:, bw, 128 * dt:128 * (dt + 1)],
                            pt_hl[:, col + kc:col + kc + 1],
                            start=(n_mm == 0), stop=(n_mm == N_MM_TOT - 1),
                        )
                        n_mm += 1
            u_sb = smp.tile([128, KD], f32)
            nc.vector.tensor_copy(u_sb[:], pu[:])
            u_hl = split_hl(u_sb[:], KD, "u_hl")

            # ---- z partial = H[:, cols_c] @ u_c (full-D, partition-major) ----
            pz = ps_z.tile([128, KZ], f32)
            n_mm = 0
            N_MM_TOT = 3 * KD * KZ
            for kc in range(KD):
                for dt in range(KZ):
                    for (bw, col) in ((kc, 0), (kc, KD), (KD + kc, 0)):
                        nc.tensor.matmul(
                            pz[:, dt:dt + 1],
                            hT[:, bw, 128 * dt:128 * (dt + 1)],
                            u_hl[:, col + kc:col + kc + 1],
                            start=(n_mm == 0), stop=(n_mm == N_MM_TOT - 1),
                        )
                        n_mm += 1

            # place this core's s_q shard via the per-core mask, add to z
            sqm = smp.tile([128, KZ], f32)
            nc.vector.tensor_tensor(
                sqm[:].rearrange("p (r k) -> p r k", k=KD),
                s_q[:].unsqueeze(1).broadcast_to([128, KZ // KD, KD]),
                mask[:].rearrange("p (r k) -> p r k", k=KD),
                mybir.AluOpType.mult,
            )
            ar2 = smp.tile([128, KZ], f32)
            nc.vector.tensor_add(ar2[:], sqm[:], pz[:])

            # ---- AllReduce: z ----
            car2_i = dram.tile([128, KZ], f32)
            car2_o = dram.tile([128, KZ], f32)
            nc.gpsimd.dma_start(car2_i[:], ar2[:])
            nc.gpsimd.collective_compute(
                "AllReduce", ADD, replica_groups=rg,
                ins=[car2_i[:].opt()], outs=[car2_o[:].opt()])
            zf = smp.tile([128, KZ], f32)
            nc.gpsimd.dma_start(zf[:], car2_o[:])

            # ---- prelu(z) = relu(z) + a * (z - relu(z)) ----
            pos = smp.tile([128, KZ], f32)
            nc.scalar.activation(pos[:], zf[:], AF.Relu)
            neg = smp.tile([128, KZ], f32)
            nc.vector.tensor_sub(neg[:], zf[:], pos[:])
            negs = smp.tile([128, KZ], f32)
            nc.vector.tensor_scalar_mul(negs[:], neg[:], ab[:])
            pzz = smp.tile([128, KZ], f32)
            nc.vector.tensor_add(pzz[:], pos[:], negs[:])
            pz_hl = split_hl(pzz[:], KZ, "pz_hl")

            # ---- y = R[rows_c] @ prelu(z), streaming R^T (hi then lo) ----
            py = ps_y.tile([1, DL], f32)
            rT_r = rT_in[:].rearrange("(n p) e -> p n e", p=128)
            n_mm = 0
            N_MM_TOT = 3 * KZ
            for i in range((2 * KZ) // RT_N):
                rt_t = rtp.tile([128, RT_N, DL], bf16)
                nc.sync.dma_start(rt_t[:], rT_r[:, RT_N * i:RT_N * (i + 1), :])
                for j in range(RT_N):
                    n = RT_N * i + j
                    is_hi = n < KZ
                    kc = n if is_hi else n - KZ
                    rhs_cols = ([kc, KZ + kc] if is_hi else [kc])
                    for col in rhs_cols:
                        nc.tensor.matmul(
                            py[:, :],
                            pz_hl[:, col:col + 1],
                            rt_t[:, j, :],
                            start=(n_mm == 0), stop=(n_mm == N_MM_TOT - 1),
                        )
                        n_mm += 1
            y_sb = smp.tile([1, DL], f32)
            nc.vector.tensor_copy(y_sb[:], py[:])
            nc.gpsimd.dma_start(y_out[:], y_sb[:])

    nc.compile()
    return nc


def _get_module(n_cores=N_CORES, d=D, m=M, l=L):  # noqa: E741
    key = (n_cores, d, m, l)
    if key not in _CACHE:
        _CACHE[key] = _build_module(n_cores, d, m, l)
    return _CACHE[key]


def _hl(x):
    """f32 array -> bf16 [hi; lo] stacked along axis 0."""
    import ml_dtypes
    bf = ml_dtypes.bfloat16
    hi = x.astype(bf)
    lo = (x - hi.astype(np.float32)).astype(bf)
    return np.concatenate([hi, lo], axis=0)


def _make_in_maps(n_cores, d, m, l, F_q, query, memory_nodes, H, R, a_out):  # noqa: E741
    f32 = np.float32
    DL = d // n_cores
    KZ = d // 128
    KD = DL // 128
    KL = l // 128
    q2d = np.ascontiguousarray(query.reshape(KL, 128).T).astype(f32, copy=False)
    q2d_hl = np.concatenate([_hl(q2d)[:128], _hl(q2d)[128:]], axis=1)
    in_maps = []
    for c in range(n_cores):
        rows = slice(DL * c, DL * (c + 1))
        mask = np.zeros((128, KZ), f32)
        mask[:, KD * c:KD * (c + 1)] = 1.0
        in_maps.append({
            "fqT": _hl(np.ascontiguousarray(F_q[rows].T)),
            "q2d": q2d_hl,
            "memd": _hl(np.ascontiguousarray(memory_nodes[rows])),
            "memT": _hl(np.ascontiguousarray(memory_nodes[rows].T)),
            "hT": _hl(np.ascontiguousarray(H[:, rows].T)),
            "rT": _hl(np.ascontiguousarray(R[rows].T)),
            "ab": np.full((128, 1), a_out, f32),
            "mask": mask,
        })
    return in_maps


class _PjrtRunner:
    """Cached jit(shard_map(bass_exec)) so repeat kernel() calls skip
    retracing/recompiling (bass_utils.run_bass_kernel_spmd rebuilds the jit
    closure every call)."""

    def __init__(self, nc, n_cores):
        import jax
        from jax.sharding import Mesh, PartitionSpec
        from jax.experimental.shard_map import shard_map
        from concourse import bass2jax
        import concourse.mybir as mybir

        bass2jax.install_neuronx_cc_hook()
        self.n_cores = n_cores
        part_name = (nc.partition_id_tensor.name
                     if nc.partition_id_tensor else None)
        in_names, out_names, out_avals = [], [], []
        for alloc in nc.m.functions[0].allocations:
            if not isinstance(alloc, mybir.MemoryLocationSet):
                continue
            name = alloc.memorylocations[0].name
            if alloc.kind == "ExternalInput":
                if name != part_name:
                    in_names.append(name)
            elif alloc.kind == "ExternalOutput":
                out_names.append(name)
                out_avals.append(jax.core.ShapedArray(
                    tuple(alloc.tensor_shape), mybir.dt.np(alloc.dtype)))
        self.in_names, self.out_names, self.out_avals = in_names, out_names, out_avals
        n_params = len(in_names)
        self.zero_outs = [np.zeros(a.shape, a.dtype) for a in out_avals]
        all_in_names = tuple(in_names + out_names)
        if part_name is not None:
            all_in_names = all_in_names + (part_name,)

        def _body(*args):
            operands = list(args)
            if part_name is not None:
                operands.append(bass2jax.partition_id_tensor())
            outs = bass2jax._bass_exec_p.bind(
                *operands,
                out_avals=tuple(out_avals),
                in_names=all_in_names,
                out_names=tuple(out_names),
                lowering_input_output_aliases=(),
                sim_require_finite=True,
                sim_require_nnan=True,
                nc=nc,
            )
            return tuple(outs)

        devices = jax.devices()[:n_cores]
        mesh = Mesh(np.asarray(devices), ("core",))
        n_out = len(out_names)
        self._fn = jax.jit(
            shard_map(
                _body, mesh=mesh,
                in_specs=(PartitionSpec("core"),) * (n_params + n_out),
                out_specs=(PartitionSpec("core"),) * n_out,
                check_rep=False,
            ),
            keep_unused=True,
        )

    def __call__(self, in_maps):
        n = self.n_cores
        concat_in = [
            np.concatenate([in_maps[c][name] for c in range(n)], axis=0)
            for name in self.in_names
        ]
        concat_zeros = [
            np.zeros((n * z.shape[0], *z.shape[1:]), z.dtype)
            for z in self.zero_outs
        ]
        out_arrs = self._fn(*concat_in, *concat_zeros)
        return [
            {name: np.asarray(out_arrs[i]).reshape(n, *self.out_avals[i].shape)[c]
             for i, name in enumerate(self.out_names)}
            for c in range(n)
        ]


_RUNNER = {}


def _get_runner():
    if "r" not in _RUNNER:
        _RUNNER["r"] = _PjrtRunner(_get_module(), N_CORES)
    return _RUNNER["r"]


def kernel(**inputs):
    f32 = np.float32
    F_q = np.asarray(inputs["F_q"], f32)
    query = np.asarray(inputs["query"], f32).reshape(-1)
    memory_nodes = np.asarray(inputs["memory_nodes"], f32)
    H = np.asarray(inputs["H"], f32)
    R = np.asarray(inputs["R"], f32)
    a_out = float(np.asarray(inputs["a_out"]).reshape(-1)[0])

    in_maps = _make_in_maps(N_CORES, D, M, L, F_q, query, memory_nodes,
                            H, R, a_out)
    results = _get_runner()(in_maps)
    y = np.concatenate([results[c]["y"].reshape(-1) for c in range(N_CORES)])
    return y.reshape(D, 1).astype(f32)


# revision 10
# speedup vs baseline: 1.3369x; 1.0068x over previous
"""EntNet forward kernel for 8 Trainium2 NeuronCores (Bass/Tile).

Math note: in the reference, the gated memory is
    mem = memory_nodes * (1 + sigmoid(...))
followed by per-column L2 normalization.  Since (1 + sigmoid(x)) > 0 is a
per-column positive scalar, it cancels exactly in the normalization, so the
gate g — and with it s_in, F_i, input, keys, U, V, W, a_mem (cand is dead in
the source already) — does not affect the output.  Live computation:

    s_q = F_q @ query[0]                         # [D]
    mn  = memory_nodes / max(||col||_2, 1e-12)   # [D, M] column-normalized
    p   = softmax(s_q^T @ mn)                    # [1, M]
    u   = mn @ p^T                               # [D]
    y   = R @ prelu(s_q + H @ u, a_out)          # [D, 1]

Sharding: D is row-sharded over 8 cores (rows_c = 512c:512c+512).  Each core
streams its shards of F_q^T, mem, mem^T (row shards), H^T (H column shard),
R^T (R row shard) — ~36 MiB/core vs 608 MiB of live bytes single-core.
Collectives: an early AllReduce of the column sum-of-squares (also absorbs
the cold-start cost of the collective machinery), a second 4 KB AllReduce of
the r = mem^T s_q partials, and a 16 KB AllReduce of z = s_q + H@u partials.

Precision: f32 PE matvecs measure ~430 ns per 128x128 weight tile on trn2,
while bf16 weight-stationary tiles pipeline at ~27 ns.  All matrices are
shipped as bf16 (hi, lo) splits — X = hi + lo with hi = bf16(X),
lo = bf16(X - hi) — and each matvec computes hi@vhi + hi@vlo + lo@vhi in
f32 PSUM (the dropped lo@lo term is O(2^-18) relative).  Same DMA bytes as
f32, ~1e-5 relative output error, >5x PE speedup.

Vectors are partition-major throughout: v[128i + p] <-> tile[p, i].
"""

import sys
import numpy as np

for _p in ("/root/.axon_site/_ro/trn_rl_repo", "/opt/trn_rl_repo"):
    if _p not in sys.path:
        sys.path.append(_p)

D, M, L = 4096, 1024, 8192
N_CORES = 8

_CACHE = {}


def _build_module(n_cores, d, m, l):  # noqa: E741
    import concourse.bacc as bacc
    import concourse.tile as tile
    import concourse.tile as tile_mod
    import concourse.mybir as mybir
    import concourse.bass_isa as bass_isa

    f32 = mybir.dt.float32
    bf16 = mybir.dt.bfloat16
    DL = d // n_cores      # local rows of D
    KD = DL // 128         # local d chunks
    KM = m // 128          # m chunks
    KZ = d // 128          # global d chunks
    KL = l // 128          # l chunks
    AF = mybir.ActivationFunctionType
    ADD = mybir.AluOpType.add
    rg = [list(range(n_cores))]

    # F_q^T hi|lo streamed in chunks of FQ_N l-tiles (8 -> 1 MiB at full size)
    FQ_N = min(8, KL)
    assert (2 * KL) % FQ_N == 0
    RT_N = min(8, KZ)
    assert (2 * KZ) % RT_N == 0

    nc = bacc.Bacc("TRN2", target_bir_lowering=False, debug=False,
                   enable_asserts=False, num_devices=n_cores)

    # hi|lo packed along the leading axis: rows [0:n] = hi, [n:2n] = lo
    fqT_in = nc.dram_tensor("fqT", [2 * l, DL], bf16, kind="ExternalInput")
    q2d_in = nc.dram_tensor("q2d", [128, 2 * KL], bf16, kind="ExternalInput")
    memd_in = nc.dram_tensor("memd", [2 * DL, m], bf16, kind="ExternalInput")
    memT_in = nc.dram_tensor("memT", [2 * m, DL], bf16, kind="ExternalInput")
    hT_in = nc.dram_tensor("hT", [2 * DL, d], bf16, kind="ExternalInput")
    rT_in = nc.dram_tensor("rT", [2 * d, DL], bf16, kind="ExternalInput")
    ab_in = nc.dram_tensor("ab", [128, 1], f32, kind="ExternalInput")
    mask_in = nc.dram_tensor("mask", [128, KZ], f32, kind="ExternalInput")
    y_out = nc.dram_tensor("y", [128, KD], f32, kind="ExternalOutput")
    hw_out = nc.dram_tensor("hw", [1, 8], f32, kind="ExternalOutput")

    with tile.TileContext(nc) as tc:
        with (
            tc.tile_pool(name="consts", bufs=1) as consts,
            tc.tile_pool(name="fq", bufs=3) as fqp,
            tc.tile_pool(name="mem", bufs=1) as memp,
            tc.tile_pool(name="big", bufs=1) as bigp,
            tc.tile_pool(name="rt", bufs=3) as rtp,
            tc.tile_pool(name="sm", bufs=1) as smp,
            tc.tile_pool(name="scr", bufs=2) as scrp,
            tc.tile_pool(name="ps_sq", bufs=1, space="PSUM") as ps_sq,
            tc.tile_pool(name="ps_r", bufs=1, space="PSUM") as ps_r,
            tc.tile_pool(name="ps_u", bufs=1, space="PSUM") as ps_u,
            tc.tile_pool(name="ps_z", bufs=1, space="PSUM") as ps_z,
            tc.tile_pool(name="ps_y", bufs=1, space="PSUM") as ps_y,
            tc.tile_pool(name="ps_h", bufs=1, space="PSUM") as ps_h,
            tc.tile_pool(name="dram", bufs=1, space="DRAM") as dram,
        ):
            # ---- ACT table warmup (Sqrt/Exp/Relu load ~1.3us each lazily) ----
            warm = consts.tile([1, 1], f32)
            nc.gpsimd.memset(warm[:], 1.0)
            w2 = consts.tile([1, 1], f32)
            nc.scalar.activation(w2[:], warm[:], AF.Square)
            nc.scalar.activation(w2[:], warm[:], AF.Sqrt)
            nc.scalar.activation(w2[:], warm[:], AF.Exp)
            nc.scalar.activation(w2[:], warm[:], AF.Relu)

            # ---- small constants ----
            q2d = consts.tile([128, 2 * KL], bf16)
            nc.gpsimd.dma_start(q2d[:], q2d_in[:])
            ab = consts.tile([128, 1], f32)
            nc.gpsimd.dma_start(ab[:], ab_in[:])
            mask = consts.tile([128, KZ], f32)
            nc.gpsimd.dma_start(mask[:], mask_in[:])

            # ---- collective warmup: absorb ncfw cold-start off the tail ----
            wsrc = consts.tile([128, 4], f32)
            nc.gpsimd.memset(wsrc[:], 0.0)
            cw_i = dram.tile([128, 4], f32)
            cw_o = dram.tile([128, 4], f32)
            nc.gpsimd.dma_start(cw_i[:], wsrc[:])
            nc.gpsimd.collective_compute(
                "AllReduce", ADD, replica_groups=rg,
                ins=[cw_i[:].opt()], outs=[cw_o[:].opt()])
            wsb = consts.tile([128, 4], f32)
            nc.gpsimd.dma_start(wsb[:], cw_o[:])

            def split_hl(src_f32, k, name):
                """f32 [128, k] -> bf16 [128, 2k] = [hi | lo]."""
                hl = smp.tile([128, 2 * k], bf16, name=name)
                nc.vector.tensor_copy(hl[:, 0:k], src_f32)
                hi_f = smp.tile([128, k], f32, name=name + "_hf")
                nc.vector.tensor_copy(hi_f[:], hl[:, 0:k])
                lo_f = smp.tile([128, k], f32, name=name + "_lf")
                nc.vector.tensor_sub(lo_f[:], src_f32, hi_f[:])
                nc.vector.tensor_copy(hl[:, k:2 * k], lo_f[:])
                return hl

            # ---- mem shards; column sum-of-squares -> early AllReduce ----
            memd = memp.tile([128, 2 * KD, m], bf16)
            memT = memp.tile([128, 2 * KM, DL], bf16)
            nc.scalar.dma_start(memT[:], memT_in[:].rearrange("(b p) e -> p b e", p=128))
            nc.scalar.dma_start(memd[:], memd_in[:].rearrange("(b p) m -> p b m", p=128))

            ss = smp.tile([128, KM], f32)
            for b in range(KM):
                full = scrp.tile([128, DL], f32, name="ssfull")
                nc.vector.tensor_add(full[:], memT[:, b, :], memT[:, KM + b, :])
                scratch = scrp.tile([128, DL], f32, name="sssq")
                nc.scalar.activation(scratch[:], full[:], AF.Square,
                                     accum_out=ss[:, b:b + 1])
            css_i = dram.tile([128, KM], f32)
            css_o = dram.tile([128, KM], f32)
            nc.gpsimd.dma_start(css_i[:], ss[:])
            nc.gpsimd.collective_compute(
                "AllReduce", ADD, replica_groups=rg,
                ins=[css_i[:].opt()], outs=[css_o[:].opt()])
            ssf = smp.tile([128, KM], f32)
            nc.gpsimd.dma_start(ssf[:], css_o[:])

            # ---- phase 1: s_q = F_q @ query, streaming F_q^T (hi then lo) ----
            psq = ps_sq.tile([128, KD], f32)
            fqT_r = fqT_in[:].rearrange("(n p) e -> p n e", p=128)
            n_mm = 0
            N_MM_TOT = 2 * KL * KD + KL * KD
            for i in range((2 * KL) // FQ_N):
                fq_t = fqp.tile([128, FQ_N, DL], bf16)
                eng = nc.sync if i % 2 == 0 else nc.scalar
                eng.dma_start(fq_t[:], fqT_r[:, FQ_N * i:FQ_N * (i + 1), :])
                for j in range(FQ_N):
                    n = FQ_N * i + j
                    is_hi = n < KL
                    nl = n if is_hi else n - KL
                    # hi tile: hi@qhi + hi@qlo ; lo tile: lo@qhi
                    rhs_cols = ([nl, KL + nl] if is_hi else [nl])
                    for dt in range(KD):
                        for col in rhs_cols:
                            nc.tensor.matmul(
                                psq[:, dt:dt + 1],
                                fq_t[:, j, 128 * dt:128 * (dt + 1)],
                                q2d[:, col:col + 1],
                                start=(n_mm == 0), stop=(n_mm == N_MM_TOT - 1),
                            )
                            n_mm += 1
            s_q = smp.tile([128, KD], f32)
            nc.vector.tensor_copy(s_q[:], psq[:])
            sq_hl = split_hl(s_q[:], KD, "sq_hl")

            # H^T resident (needed right after softmax)
            hT = bigp.tile([128, 2 * KD, d], bf16)
            hT_r = hT_in[:].rearrange("(b p) e -> p b e", p=128)
            nc.sync.dma_start(hT[:, 0:KD, :], hT_r[:, 0:KD, :])
            nc.scalar.dma_start(hT[:, KD:2 * KD, :], hT_r[:, KD:2 * KD, :])

            # ---- r = mem^T @ s_q (local-d partial) ----
            pr = ps_r.tile([128, KM], f32)
            n_mm = 0
            N_MM_TOT = 3 * KM * KD
            for mt in range(KM):
                for kc in range(KD):
                    for (bw, col) in ((kc, 0), (kc, KD), (KD + kc, 0)):
                        last_r_mm = nc.tensor.matmul(
                            pr[:, mt:mt + 1],
                            memd[:, bw, 128 * mt:128 * (mt + 1)],
                            sq_hl[:, col + kc:col + kc + 1],
                            start=(n_mm == 0), stop=(n_mm == N_MM_TOT - 1),
                        )
                        n_mm += 1
            r_sb = smp.tile([128, KM], f32)
            r_copy = nc.vector.tensor_copy(r_sb[:], pr[:])

            # PE heater: keep HAM warm through the r-AllReduce wait
            HW_N = min(512, DL)
            ph = ps_h.tile([1, HW_N], f32)
            h1 = []
            for k in range(30):
                h1.append(nc.tensor.matmul(
                    ph[:, :], memT[:, 0, k:k + 1], memT[:, 0, 0:HW_N],
                    start=(k == 0), stop=(k == 29)))
            tile_mod.add_dep_helper(h1[0].ins, last_r_mm.ins, sync=False,
                                    reason="heater after r")

            # ---- AllReduce: r ----
            cr_i = dram.tile([128, KM], f32)
            cr_o = dram.tile([128, KM], f32)
            nc.gpsimd.dma_start(cr_i[:], r_sb[:])
            nc.gpsimd.collective_compute(
                "AllReduce", ADD, replica_groups=rg,
                ins=[cr_i[:].opt()], outs=[cr_o[:].opt()])
            rf = smp.tile([128, KM], f32)
            nc.gpsimd.dma_start(rf[:], cr_o[:])

            # ---- softmax (partition-major), fold 1/denom into p ----
            dn = smp.tile([128, KM], f32)
            nc.scalar.activation(dn[:], ssf[:], AF.Sqrt)
            dnm = smp.tile([128, KM], f32)
            dnm_op = nc.vector.tensor_scalar_max(dnm[:], dn[:], 1e-12)
            tile_mod.add_dep_helper(dnm_op.ins, r_copy.ins, sync=False,
                                    reason="softmax DVE after s_q/r path")
            rdn = smp.tile([128, KM], f32)
            nc.vector.reciprocal(rdn[:], dnm[:])
            t = smp.tile([128, KM], f32)
            nc.vector.tensor_mul(t[:], rf[:], rdn[:])
            tm = smp.tile([128, 1], f32)
            nc.vector.tensor_reduce(tm[:], t[:], mybir.AxisListType.X,
                                    mybir.AluOpType.max)
            tmb = smp.tile([128, 1], f32)
            nc.gpsimd.partition_all_reduce(tmb[:], tm[:], 128,
                                           bass_isa.ReduceOp.max)
            negmx = smp.tile([128, 1], f32)
            nc.vector.tensor_scalar_mul(negmx[:], tmb[:], -1.0)
            e = smp.tile([128, KM], f32)
            esum = smp.tile([128, 1], f32)
            nc.scalar.activation(e[:], t[:], AF.Exp, bias=negmx[:],
                                 accum_out=esum[:])
            esb = smp.tile([128, 1], f32)
            nc.gpsimd.partition_all_reduce(esb[:], esum[:], 128,
                                           bass_isa.ReduceOp.add)
            sd = smp.tile([128, KM], f32)
            nc.vector.tensor_scalar_mul(sd[:], dnm[:], esb[:])
            rsd = smp.tile([128, KM], f32)
            nc.vector.reciprocal(rsd[:], sd[:])
            pt = smp.tile([128, KM], f32)
            nc.vector.tensor_mul(pt[:], e[:], rsd[:])
            pt_hl = split_hl(pt[:], KM, "pt_hl")

            # ---- u = mem @ (p/denom), local rows ----
            pu = ps_u.tile([128, KD], f32)
            n_mm = 0
            N_MM_TOT = 3 * KD * KM
            for dt in range(KD):
                for kc in range(KM):
                    for (bw, col) in ((kc, 0), (kc, KM), (KM + kc, 0)):
                        nc.tensor.matmul(
                            pu[:, dt:dt + 1],
                            memT[:, bw, 128 * dt:128 * (dt + 1)],
                            pt_hl[:, col + kc:col + kc + 1],
                            start=(n_mm == 0), stop=(n_mm == N_MM_TOT - 1),
                        )
                        n_mm += 1
            u_sb = smp.tile([128, KD], f32)
            nc.vector.tensor_copy(u_sb[:], pu[:])
            u_hl = split_hl(u_sb[:], KD, "u_hl")

            # ---- z partial = H[:, cols_c] @ u_c (full-D, partition-major) ----
            pz = ps_z.tile([128, KZ], f32)
            n_mm = 0
            N_MM_TOT = 3 * KD * KZ
            for kc in range(KD):
                for dt in range(KZ):
                    for (bw, col) in ((kc, 0), (kc, KD), (KD + kc, 0)):
                        last_z_mm = nc.tensor.matmul(
                            pz[:, dt:dt + 1],
                            hT[:, bw, 128 * dt:128 * (dt + 1)],
                            u_hl[:, col + kc:col + kc + 1],
                            start=(n_mm == 0), stop=(n_mm == N_MM_TOT - 1),
                        )
                        n_mm += 1
            h2 = []
            for k in range(52):
                h2.append(nc.tensor.matmul(
                    ph[:, :], memT[:, 0, k:k + 1], memT[:, 0, 0:HW_N],
                    start=(k == 0), stop=(k == 51)))
            tile_mod.add_dep_helper(h2[0].ins, last_z_mm.ins, sync=False,
                                    reason="heater after z")

            # place this core's s_q shard via the per-core mask, add to z
            sqm = smp.tile([128, KZ], f32)
            nc.vector.tensor_tensor(
                sqm[:].rearrange("p (r k) -> p r k", k=KD),
                s_q[:].unsqueeze(1).broadcast_to([128, KZ // KD, KD]),
                mask[:].rearrange("p (r k) -> p r k", k=KD),
                mybir.AluOpType.mult,
            )
            ar2 = smp.tile([128, KZ], f32)
            nc.vector.tensor_add(ar2[:], sqm[:], pz[:])

            # ---- AllReduce: z ----
            car2_i = dram.tile([128, KZ], f32)
            car2_o = dram.tile([128, KZ], f32)
            nc.gpsimd.dma_start(car2_i[:], ar2[:])
            nc.gpsimd.collective_compute(
                "AllReduce", ADD, replica_groups=rg,
                ins=[car2_i[:].opt()], outs=[car2_o[:].opt()])
            zf = smp.tile([128, KZ], f32)
            nc.gpsimd.dma_start(zf[:], car2_o[:])

            # ---- prelu(z) = relu(z) + a * (z - relu(z)) ----
            pos = smp.tile([128, KZ], f32)
            nc.scalar.activation(pos[:], zf[:], AF.Relu)
            neg = smp.tile([128, KZ], f32)
            nc.vector.tensor_sub(neg[:], zf[:], pos[:])
            negs = smp.tile([128, KZ], f32)
            nc.vector.tensor_scalar_mul(negs[:], neg[:], ab[:])
            pzz = smp.tile([128, KZ], f32)
            nc.vector.tensor_add(pzz[:], pos[:], negs[:])
            pz_hl = split_hl(pzz[:], KZ, "pz_hl")

            # ---- y = R[rows_c] @ prelu(z): ws-bf16, partition-major out ----
            py = ps_y.tile([128, KD], f32)
            rT_r = rT_in[:].rearrange("(n p) e -> p n e", p=128)
            n_mm = 0
            N_MM_TOT = 3 * KZ * KD
            for i in range((2 * KZ) // RT_N):
                rt_t = rtp.tile([128, RT_N, DL], bf16)
                eng = nc.sync if i % 2 == 0 else nc.scalar
                eng.dma_start(rt_t[:], rT_r[:, RT_N * i:RT_N * (i + 1), :])
                for j in range(RT_N):
                    n = RT_N * i + j
                    is_hi = n < KZ
                    kc = n if is_hi else n - KZ
                    rhs_cols = ([kc, KZ + kc] if is_hi else [kc])
                    for dt in range(KD):
                        for col in rhs_cols:
                            nc.tensor.matmul(
                                py[:, dt:dt + 1],
                                rt_t[:, j, 128 * dt:128 * (dt + 1)],
                                pz_hl[:, col:col + 1],
                                start=(n_mm == 0), stop=(n_mm == N_MM_TOT - 1),
                            )
                            n_mm += 1
            y_sb = smp.tile([128, KD], f32)
            nc.vector.tensor_copy(y_sb[:], py[:])
            nc.gpsimd.dma_start(y_out[:], y_sb[:])

            # consume heater + warmup results (anti-DCE) via dummy output
            hw_sb = smp.tile([1, 8], f32)
            nc.vector.tensor_copy(hw_sb[:, 0:4], ph[0:1, 0:4])
            nc.vector.tensor_copy(hw_sb[:, 4:8], wsb[0:1, 0:4])
            nc.gpsimd.dma_start(hw_out[:], hw_sb[:])

    nc.compile()
    return nc


def _get_module(n_cores=N_CORES, d=D, m=M, l=L):  # noqa: E741
    key = (n_cores, d, m, l)
    if key not in _CACHE:
        _CACHE[key] = _build_module(n_cores, d, m, l)
    return _CACHE[key]


def _hl(x):
    """f32 array -> bf16 [hi; lo] stacked along axis 0."""
    import ml_dtypes
    bf = ml_dtypes.bfloat16
    hi = x.astype(bf)
    lo = (x - hi.astype(np.float32)).astype(bf)
    return np.concatenate([hi, lo], axis=0)


def _make_in_maps(n_cores, d, m, l, F_q, query, memory_nodes, H, R, a_out):  # noqa: E741
    f32 = np.float32
    DL = d // n_cores
    KZ = d // 128
    KD = DL // 128
    KL = l // 128
    q2d = np.ascontiguousarray(query.reshape(KL, 128).T).astype(f32, copy=False)
    q2d_hl = np.concatenate([_hl(q2d)[:128], _hl(q2d)[128:]], axis=1)
    in_maps = []
    for c in range(n_cores):
        rows = slice(DL * c, DL * (c + 1))
        mask = np.zeros((128, KZ), f32)
        mask[:, KD * c:KD * (c + 1)] = 1.0
        in_maps.append({
            "fqT": _hl(np.ascontiguousarray(F_q[rows].T)),
            "q2d": q2d_hl,
            "memd": _hl(np.ascontiguousarray(memory_nodes[rows])),
            "memT": _hl(np.ascontiguousarray(memory_nodes[rows].T)),
            "hT": _hl(np.ascontiguousarray(H[:, rows].T)),
            "rT": _hl(np.ascontiguousarray(R[rows].T)),
            "ab": np.full((128, 1), a_out, f32),
            "mask": mask,
        })
    return in_maps


class _PjrtRunner:
    """Cached jit(shard_map(bass_exec)) so repeat kernel() calls skip
    retracing/recompiling (bass_utils.run_bass_kernel_spmd rebuilds the jit
    closure every call)."""

    def __init__(self, nc, n_cores):
        import jax
        from jax.sharding import Mesh, PartitionSpec
        from jax.experimental.shard_map import shard_map
        from concourse import bass2jax
        import concourse.mybir as mybir

        bass2jax.install_neuronx_cc_hook()
        self.n_cores = n_cores
        part_name = (nc.partition_id_tensor.name
                     if nc.partition_id_tensor else None)
        in_names, out_names, out_avals = [], [], []
        for alloc in nc.m.functions[0].allocations:
            if not isinstance(alloc, mybir.MemoryLocationSet):
                continue
            name = alloc.memorylocations[0].name
            if alloc.kind == "ExternalInput":
                if name != part_name:
                    in_names.append(name)
            elif alloc.kind == "ExternalOutput":
                out_names.append(name)
                out_avals.append(jax.core.ShapedArray(
                    tuple(alloc.tensor_shape), mybir.dt.np(alloc.dtype)))
        self.in_names, self.out_names, self.out_avals = in_names, out_names, out_avals
        n_params = len(in_names)
        self.zero_outs = [np.zeros(a.shape, a.dtype) for a in out_avals]
        all_in_names = tuple(in_names + out_names)
        if part_name is not None:
            all_in_names = all_in_names + (part_name,)

        def _body(*args):
            operands = list(args)
            if part_name is not None:
                operands.append(bass2jax.partition_id_tensor())
            outs = bass2jax._bass_exec_p.bind(
                *operands,
                out_avals=tuple(out_avals),
                in_names=all_in_names,
                out_names=tuple(out_names),
                lowering_input_output_aliases=(),
                sim_require_finite=True,
                sim_require_nnan=True,
                nc=nc,
            )
            return tuple(outs)

        devices = jax.devices()[:n_cores]
        mesh = Mesh(np.asarray(devices), ("core",))
        n_out = len(out_names)
        self._fn = jax.jit(
            shard_map(
                _body, mesh=mesh,
                in_specs=(PartitionSpec("core"),) * (n_params + n_out),
                out_specs=(PartitionSpec("core"),) * n_out,
                check_rep=False,
            ),
            keep_unused=True,
        )

    def __call__(self, in_maps):
        n = self.n_cores
        concat_in = [
            np.concatenate([in_maps[c][name] for c in range(n)], axis=0)
            for name in self.in_names
        ]
        concat_zeros = [
            np.zeros((n * z.shape[0], *z.shape[1:]), z.dtype)
            for z in self.zero_outs
        ]
        out_arrs = self._fn(*concat_in, *concat_zeros)
        return [
            {name: np.asarray(out_arrs[i]).reshape(n, *self.out_avals[i].shape)[c]
             for i, name in enumerate(self.out_names)}
            for c in range(n)
        ]


_RUNNER = {}


def _get_runner():
    if "r" not in _RUNNER:
        _RUNNER["r"] = _PjrtRunner(_get_module(), N_CORES)
    return _RUNNER["r"]


def kernel(**inputs):
    f32 = np.float32
    F_q = np.asarray(inputs["F_q"], f32)
    query = np.asarray(inputs["query"], f32).reshape(-1)
    memory_nodes = np.asarray(inputs["memory_nodes"], f32)
    H = np.asarray(inputs["H"], f32)
    R = np.asarray(inputs["R"], f32)
    a_out = float(np.asarray(inputs["a_out"]).reshape(-1)[0])

    in_maps = _make_in_maps(N_CORES, D, M, L, F_q, query, memory_nodes,
                            H, R, a_out)
    results = _get_runner()(in_maps)
    y = np.concatenate(
        [np.ascontiguousarray(results[c]["y"].T).reshape(-1)
         for c in range(N_CORES)])
    return y.reshape(D, 1).astype(f32)


# revision 13
# speedup vs baseline: 1.4332x; 1.0721x over previous
"""EntNet forward kernel for 8 Trainium2 NeuronCores (Bass/Tile).

Math note: in the reference, the gated memory is
    mem = memory_nodes * (1 + sigmoid(...))
followed by per-column L2 normalization.  Since (1 + sigmoid(x)) > 0 is a
per-column positive scalar, it cancels exactly in the normalization, so the
gate g — and with it s_in, F_i, input, keys, U, V, W, a_mem (cand is dead in
the source already) — does not affect the output.  Live computation:

    s_q = F_q @ query[0]                         # [D]
    mn  = memory_nodes / max(||col||_2, 1e-12)   # [D, M] column-normalized
    p   = softmax(s_q^T @ mn)                    # [1, M]
    u   = mn @ p^T                               # [D]
    y   = R @ prelu(s_q + H @ u, a_out)          # [D, 1]

Sharding: D is row-sharded over 8 cores (rows_c = 512c:512c+512).  Each core
streams its shards of F_q^T, mem, mem^T (row shards), H^T (H column shard),
R^T (R row shard) — ~36 MiB/core vs 608 MiB of live bytes single-core.
Collectives: an early AllReduce of the column sum-of-squares (also absorbs
the cold-start cost of the collective machinery), a second 4 KB AllReduce of
the r = mem^T s_q partials, and a 16 KB AllReduce of z = s_q + H@u partials.

Precision: f32 PE matvecs measure ~430 ns per 128x128 weight tile on trn2,
while bf16 weight-stationary tiles pipeline at ~27 ns.  All matrices are
shipped as bf16 (hi, lo) splits — X = hi + lo with hi = bf16(X),
lo = bf16(X - hi) — and each matvec computes hi@vhi + hi@vlo + lo@vhi in
f32 PSUM (the dropped lo@lo term is O(2^-18) relative).  Same DMA bytes as
f32, ~1e-5 relative output error, >5x PE speedup.

Vectors are partition-major throughout: v[128i + p] <-> tile[p, i].
"""

import sys
import numpy as np

for _p in ("/root/.axon_site/_ro/trn_rl_repo", "/opt/trn_rl_repo"):
    if _p not in sys.path:
        sys.path.append(_p)

D, M, L = 4096, 1024, 8192
N_CORES = 8

_CACHE = {}


def _build_module(n_cores, d, m, l):  # noqa: E741
    import concourse.bacc as bacc
    import concourse.tile as tile
    import concourse.tile as tile_mod
    import concourse.mybir as mybir
    import concourse.bass_isa as bass_isa

    f32 = mybir.dt.float32
    bf16 = mybir.dt.bfloat16
    DL = d // n_cores      # local rows of D
    KD = DL // 128         # local d chunks
    KM = m // 128          # m chunks
    KZ = d // 128          # global d chunks
    KL = l // 128          # l chunks
    AF = mybir.ActivationFunctionType
    ADD = mybir.AluOpType.add
    rg = [list(range(n_cores))]

    # F_q^T hi|lo streamed in chunks of FQ_N l-tiles (8 -> 1 MiB at full size)
    FQ_N = min(8, KL)
    assert (2 * KL) % FQ_N == 0
    RT_N = min(8, KZ)
    assert (2 * KZ) % RT_N == 0

    nc = bacc.Bacc("TRN2", target_bir_lowering=False, debug=False,
                   enable_asserts=False, num_devices=n_cores)

    # hi|lo packed along the leading axis: rows [0:n] = hi, [n:2n] = lo
    fqT_in = nc.dram_tensor("fqT", [2 * l, DL], bf16, kind="ExternalInput")
    q2d_in = nc.dram_tensor("q2d", [128, 2 * KL], bf16, kind="ExternalInput")
    memd_in = nc.dram_tensor("memd", [2 * DL, m], bf16, kind="ExternalInput")
    memT_in = nc.dram_tensor("memT", [2 * m, DL], bf16, kind="ExternalInput")
    hT_in = nc.dram_tensor("hT", [2 * DL, d], bf16, kind="ExternalInput")
    rT_in = nc.dram_tensor("rT", [2 * d, DL], bf16, kind="ExternalInput")
    ab_in = nc.dram_tensor("ab", [128, 1], f32, kind="ExternalInput")
    mask_in = nc.dram_tensor("mask", [128, KZ], f32, kind="ExternalInput")
    y_out = nc.dram_tensor("y", [128, KD], f32, kind="ExternalOutput")
    hw_out = nc.dram_tensor("hw", [1, 8], f32, kind="ExternalOutput")

    with tile.TileContext(nc) as tc:
        with (
            tc.tile_pool(name="consts", bufs=1) as consts,
            tc.tile_pool(name="fq", bufs=3) as fqp,
            tc.tile_pool(name="mem", bufs=1) as memp,
            tc.tile_pool(name="big", bufs=1) as bigp,
            tc.tile_pool(name="rt", bufs=6) as rtp,
            tc.tile_pool(name="sm", bufs=1) as smp,
            tc.tile_pool(name="scr", bufs=2) as scrp,
            tc.tile_pool(name="ps_sq", bufs=1, space="PSUM") as ps_sq,
            tc.tile_pool(name="ps_r", bufs=1, space="PSUM") as ps_r,
            tc.tile_pool(name="ps_u", bufs=1, space="PSUM") as ps_u,
            tc.tile_pool(name="ps_z", bufs=1, space="PSUM") as ps_z,
            tc.tile_pool(name="ps_y", bufs=1, space="PSUM") as ps_y,
            tc.tile_pool(name="ps_h", bufs=1, space="PSUM") as ps_h,
            tc.tile_pool(name="dram", bufs=1, space="DRAM") as dram,
        ):
            # ---- ACT table warmup (Sqrt/Exp/Relu load ~1.3us each lazily) ----
            warm = consts.tile([1, 1], f32)
            nc.gpsimd.memset(warm[:], 1.0)
            w2 = consts.tile([1, 1], f32)
            nc.scalar.activation(w2[:], warm[:], AF.Square)
            nc.scalar.activation(w2[:], warm[:], AF.Sqrt)
            nc.scalar.activation(w2[:], warm[:], AF.Exp)
            nc.scalar.activation(w2[:], warm[:], AF.Relu)

            # ---- small constants ----
            q2d = consts.tile([128, 2 * KL], bf16)
            nc.gpsimd.dma_start(q2d[:], q2d_in[:])
            ab = consts.tile([128, 1], f32)
            nc.gpsimd.dma_start(ab[:], ab_in[:])
            mask = consts.tile([128, KZ], f32)
            nc.gpsimd.dma_start(mask[:], mask_in[:])


            def split_hl(src_f32, k, name):
                """f32 [128, k] -> bf16 [128, 2k] = [hi | lo]."""
                hl = smp.tile([128, 2 * k], bf16, name=name)
                nc.vector.tensor_copy(hl[:, 0:k], src_f32)
                hi_f = smp.tile([128, k], f32, name=name + "_hf")
                nc.vector.tensor_copy(hi_f[:], hl[:, 0:k])
                lo_f = smp.tile([128, k], f32, name=name + "_lf")
                nc.vector.tensor_sub(lo_f[:], src_f32, hi_f[:])
                nc.vector.tensor_copy(hl[:, k:2 * k], lo_f[:])
                return hl

            # ---- mem shards; column sum-of-squares -> early AllReduce ----
            memd = memp.tile([128, 2 * KD, m], bf16)
            memT = memp.tile([128, 2 * KM, DL], bf16)
            nc.scalar.dma_start(memT[:], memT_in[:].rearrange("(b p) e -> p b e", p=128))
            nc.scalar.dma_start(memd[:], memd_in[:].rearrange("(b p) m -> p b m", p=128))

            ss = smp.tile([128, KM], f32)
            for b in range(KM):
                full = scrp.tile([128, DL], f32, name="ssfull")
                nc.vector.tensor_add(full[:], memT[:, b, :], memT[:, KM + b, :])
                scratch = scrp.tile([128, DL], f32, name="sssq")
                nc.scalar.activation(scratch[:], full[:], AF.Square,
                                     accum_out=ss[:, b:b + 1])
            css_i = dram.tile([128, KM], f32)
            css_o = dram.tile([128, KM], f32)
            nc.gpsimd.dma_start(css_i[:], ss[:])
            nc.gpsimd.collective_compute(
                "AllReduce", ADD, replica_groups=rg,
                ins=[css_i[:].opt()], outs=[css_o[:].opt()])
            ssf = smp.tile([128, KM], f32)
            nc.gpsimd.dma_start(ssf[:], css_o[:])

            # ---- phase 1: s_q = F_q @ query, streaming F_q^T (hi then lo) ----
            psq = ps_sq.tile([128, KD], f32)
            fqT_r = fqT_in[:].rearrange("(n p) e -> p n e", p=128)
            n_mm = 0
            N_MM_TOT = 2 * KL * KD + KL * KD
            for i in range((2 * KL) // FQ_N):
                fq_t = fqp.tile([128, FQ_N, DL], bf16)
                eng = nc.sync if i % 2 == 0 else nc.scalar
                eng.dma_start(fq_t[:], fqT_r[:, FQ_N * i:FQ_N * (i + 1), :])
                for j in range(FQ_N):
                    n = FQ_N * i + j
                    is_hi = n < KL
                    nl = n if is_hi else n - KL
                    # hi tile: hi@qhi + hi@qlo ; lo tile: lo@qhi
                    rhs_cols = ([nl, KL + nl] if is_hi else [nl])
                    for dt in range(KD):
                        for col in rhs_cols:
                            nc.tensor.matmul(
                                psq[:, dt:dt + 1],
                                fq_t[:, j, 128 * dt:128 * (dt + 1)],
                                q2d[:, col:col + 1],
                                start=(n_mm == 0), stop=(n_mm == N_MM_TOT - 1),
                            )
                            n_mm += 1
            s_q = smp.tile([128, KD], f32)
            nc.vector.tensor_copy(s_q[:], psq[:])
            sq_hl = split_hl(s_q[:], KD, "sq_hl")

            # H^T resident (needed right after softmax)
            hT = bigp.tile([128, 2 * KD, d], bf16)
            hT_r = hT_in[:].rearrange("(b p) e -> p b e", p=128)
            nc.sync.dma_start(hT[:, 0:KD, :], hT_r[:, 0:KD, :])
            nc.scalar.dma_start(hT[:, KD:2 * KD, :], hT_r[:, KD:2 * KD, :])

            # ---- r = mem^T @ s_q (local-d partial) ----
            pr = ps_r.tile([128, KM], f32)
            n_mm = 0
            N_MM_TOT = 3 * KM * KD
            for mt in range(KM):
                for kc in range(KD):
                    for (bw, col) in ((kc, 0), (kc, KD), (KD + kc, 0)):
                        last_r_mm = nc.tensor.matmul(
                            pr[:, mt:mt + 1],
                            memd[:, bw, 128 * mt:128 * (mt + 1)],
                            sq_hl[:, col + kc:col + kc + 1],
                            start=(n_mm == 0), stop=(n_mm == N_MM_TOT - 1),
                        )
                        n_mm += 1
            r_sb = smp.tile([128, KM], f32)
            r_copy = nc.vector.tensor_copy(r_sb[:], pr[:])

            # PE heater: keep HAM warm through the r-AllReduce wait
            HW_N = min(512, DL)
            ph = ps_h.tile([1, HW_N], f32)
            h1 = []
            for k in range(30):
                h1.append(nc.tensor.matmul(
                    ph[:, :], memT[:, 0, k:k + 1], memT[:, 0, 0:HW_N],
                    start=(k == 0), stop=(k == 29)))
            tile_mod.add_dep_helper(h1[0].ins, last_r_mm.ins, sync=False,
                                    reason="heater after r")

            # ---- AllReduce: r ----
            cr_i = dram.tile([128, KM], f32)
            cr_o = dram.tile([128, KM], f32)
            nc.gpsimd.dma_start(cr_i[:], r_sb[:])
            nc.gpsimd.collective_compute(
                "AllReduce", ADD, replica_groups=rg,
                ins=[cr_i[:].opt()], outs=[cr_o[:].opt()])
            rf = smp.tile([128, KM], f32)
            nc.gpsimd.dma_start(rf[:], cr_o[:])

            # ---- softmax (partition-major), fold 1/denom into p ----
            dn = smp.tile([128, KM], f32)
            nc.scalar.activation(dn[:], ssf[:], AF.Sqrt)
            dnm = smp.tile([128, KM], f32)
            dnm_op = nc.vector.tensor_scalar_max(dnm[:], dn[:], 1e-12)
            tile_mod.add_dep_helper(dnm_op.ins, r_copy.ins, sync=False,
                                    reason="softmax DVE after s_q/r path")
            rdn = smp.tile([128, KM], f32)
            nc.vector.reciprocal(rdn[:], dnm[:])
            t = smp.tile([128, KM], f32)
            nc.vector.tensor_mul(t[:], rf[:], rdn[:])
            tm = smp.tile([128, 1], f32)
            nc.vector.tensor_reduce(tm[:], t[:], mybir.AxisListType.X,
                                    mybir.AluOpType.max)
            tmb = smp.tile([128, 1], f32)
            nc.gpsimd.partition_all_reduce(tmb[:], tm[:], 128,
                                           bass_isa.ReduceOp.max)
            negmx = smp.tile([128, 1], f32)
            nc.vector.tensor_scalar_mul(negmx[:], tmb[:], -1.0)
            e = smp.tile([128, KM], f32)
            esum = smp.tile([128, 1], f32)
            nc.scalar.activation(e[:], t[:], AF.Exp, bias=negmx[:],
                                 accum_out=esum[:])
            esb = smp.tile([128, 1], f32)
            nc.gpsimd.partition_all_reduce(esb[:], esum[:], 128,
                                           bass_isa.ReduceOp.add)
            sd = smp.tile([128, KM], f32)
            nc.vector.tensor_scalar_mul(sd[:], dnm[:], esb[:])
            rsd = smp.tile([128, KM], f32)
            nc.vector.reciprocal(rsd[:], sd[:])
            pt = smp.tile([128, KM], f32)
            nc.vector.tensor_mul(pt[:], e[:], rsd[:])
            pt_hl = split_hl(pt[:], KM, "pt_hl")

            # ---- u = mem @ (p/denom), local rows ----
            pu = ps_u.tile([128, KD], f32)
            n_mm = 0
            N_MM_TOT = 3 * KD * KM
            for dt in range(KD):
                for kc in range(KM):
                    for (bw, col) in ((kc, 0), (kc, KM), (KM + kc, 0)):
                        nc.tensor.matmul(
                            pu[:, dt:dt + 1],
                            memT[:, bw, 128 * dt:128 * (dt + 1)],
                            pt_hl[:, col + kc:col + kc + 1],
                            start=(n_mm == 0), stop=(n_mm == N_MM_TOT - 1),
                        )
                        n_mm += 1
            u_sb = smp.tile([128, KD], f32)
            nc.vector.tensor_copy(u_sb[:], pu[:])
            u_hl = split_hl(u_sb[:], KD, "u_hl")

            # ---- z partial = H[:, cols_c] @ u_c (full-D, partition-major) ----
            pz = ps_z.tile([128, KZ], f32)
            n_mm = 0
            N_MM_TOT = 3 * KD * KZ
            for kc in range(KD):
                for dt in range(KZ):
                    for (bw, col) in ((kc, 0), (kc, KD), (KD + kc, 0)):
                        last_z_mm = nc.tensor.matmul(
                            pz[:, dt:dt + 1],
                            hT[:, bw, 128 * dt:128 * (dt + 1)],
                            u_hl[:, col + kc:col + kc + 1],
                            start=(n_mm == 0), stop=(n_mm == N_MM_TOT - 1),
                        )
                        n_mm += 1
            h2 = []
            for k in range(52):
                h2.append(nc.tensor.matmul(
                    ph[:, :], memT[:, 0, k:k + 1], memT[:, 0, 0:HW_N],
                    start=(k == 0), stop=(k == 51)))
            tile_mod.add_dep_helper(h2[0].ins, last_z_mm.ins, sync=False,
                                    reason="heater after z")

            # place this core's s_q shard via the per-core mask, add to z
            sqm = smp.tile([128, KZ], f32)
            nc.vector.tensor_tensor(
                sqm[:].rearrange("p (r k) -> p r k", k=KD),
                s_q[:].unsqueeze(1).broadcast_to([128, KZ // KD, KD]),
                mask[:].rearrange("p (r k) -> p r k", k=KD),
                mybir.AluOpType.mult,
            )
            ar2 = smp.tile([128, KZ], f32)
            nc.vector.tensor_add(ar2[:], sqm[:], pz[:])

            # ---- AllReduce: z ----
            car2_i = dram.tile([128, KZ], f32)
            car2_o = dram.tile([128, KZ], f32)
            nc.gpsimd.dma_start(car2_i[:], ar2[:])
            nc.gpsimd.collective_compute(
                "AllReduce", ADD, replica_groups=rg,
                ins=[car2_i[:].opt()], outs=[car2_o[:].opt()])
            zf = smp.tile([128, KZ], f32)
            nc.gpsimd.dma_start(zf[:], car2_o[:])

            # ---- prelu(z) = relu(z) + a * (z - relu(z)) ----
            pos = smp.tile([128, KZ], f32)
            nc.scalar.activation(pos[:], zf[:], AF.Relu)
            neg = smp.tile([128, KZ], f32)
            nc.vector.tensor_sub(neg[:], zf[:], pos[:])
            negs = smp.tile([128, KZ], f32)
            nc.vector.tensor_scalar_mul(negs[:], neg[:], ab[:])
            pzz = smp.tile([128, KZ], f32)
            nc.vector.tensor_add(pzz[:], pos[:], negs[:])
            pz_hl = split_hl(pzz[:], KZ, "pz_hl")

            # ---- y = R[rows_c] @ prelu(z): ws-bf16, partition-major out ----
            py = ps_y.tile([128, KD], f32)
            rT_r = rT_in[:].rearrange("(n p) e -> p n e", p=128)
            n_mm = 0
            N_MM_TOT = 3 * KZ * KD
            for i in range((2 * KZ) // RT_N):
                rt_t = rtp.tile([128, RT_N, DL], bf16)
                eng = nc.sync if i % 2 == 0 else nc.scalar
                eng.dma_start(rt_t[:], rT_r[:, RT_N * i:RT_N * (i + 1), :])
                for j in range(RT_N):
                    n = RT_N * i + j
                    is_hi = n < KZ
                    kc = n if is_hi else n - KZ
                    rhs_cols = ([kc, KZ + kc] if is_hi else [kc])
                    for dt in range(KD):
                        for col in rhs_cols:
                            nc.tensor.matmul(
                                py[:, dt:dt + 1],
                                rt_t[:, j, 128 * dt:128 * (dt + 1)],
                                pz_hl[:, col:col + 1],
                                start=(n_mm == 0), stop=(n_mm == N_MM_TOT - 1),
                            )
                            n_mm += 1
            y_sb = smp.tile([128, KD], f32)
            nc.vector.tensor_copy(y_sb[:], py[:])
            nc.gpsimd.dma_start(y_out[:], y_sb[:])

            # consume heater + warmup results (anti-DCE) via dummy output
            hw_sb = smp.tile([1, 8], f32)
            nc.vector.memset(hw_sb[:], 0.0)
            nc.vector.tensor_copy(hw_sb[:, 0:4], ph[0:1, 0:4])
            nc.vector.tensor_copy(hw_sb[:, 4:4 + min(4, KM)], ssf[0:1, 0:min(4, KM)])
            nc.gpsimd.dma_start(hw_out[:], hw_sb[:])

    nc.compile()
    return nc


def _get_module(n_cores=N_CORES, d=D, m=M, l=L):  # noqa: E741
    key = (n_cores, d, m, l)
    if key not in _CACHE:
        _CACHE[key] = _build_module(n_cores, d, m, l)
    return _CACHE[key]


def _hl(x):
    """f32 array -> bf16 [hi; lo] stacked along axis 0."""
    import ml_dtypes
    bf = ml_dtypes.bfloat16
    hi = x.astype(bf)
    lo = (x - hi.astype(np.float32)).astype(bf)
    return np.concatenate([hi, lo], axis=0)


def _make_in_maps(n_cores, d, m, l, F_q, query, memory_nodes, H, R, a_out):  # noqa: E741
    f32 = np.float32
    DL = d // n_cores
    KZ = d // 128
    KD = DL // 128
    KL = l // 128
    q2d = np.ascontiguousarray(query.reshape(KL, 128).T).astype(f32, copy=False)
    q2d_hl = np.concatenate([_hl(q2d)[:128], _hl(q2d)[128:]], axis=1)
    in_maps = []
    for c in range(n_cores):
        rows = slice(DL * c, DL * (c + 1))
        mask = np.zeros((128, KZ), f32)
        mask[:, KD * c:KD * (c + 1)] = 1.0
        in_maps.append({
            "fqT": _hl(np.ascontiguousarray(F_q[rows].T)),
            "q2d": q2d_hl,
            "memd": _hl(np.ascontiguousarray(memory_nodes[rows])),
            "memT": _hl(np.ascontiguousarray(memory_nodes[rows].T)),
            "hT": _hl(np.ascontiguousarray(H[:, rows].T)),
            "rT": _hl(np.ascontiguousarray(R[rows].T)),
            "ab": np.full((128, 1), a_out, f32),
            "mask": mask,
        })
    return in_maps


class _PjrtRunner:
    """Cached jit(shard_map(bass_exec)) so repeat kernel() calls skip
    retracing/recompiling (bass_utils.run_bass_kernel_spmd rebuilds the jit
    closure every call)."""

    def __init__(self, nc, n_cores):
        import jax
        from jax.sharding import Mesh, PartitionSpec
        from jax.experimental.shard_map import shard_map
        from concourse import bass2jax
        import concourse.mybir as mybir

        bass2jax.install_neuronx_cc_hook()
        self.n_cores = n_cores
        part_name = (nc.partition_id_tensor.name
                     if nc.partition_id_tensor else None)
        in_names, out_names, out_avals = [], [], []
        for alloc in nc.m.functions[0].allocations:
            if not isinstance(alloc, mybir.MemoryLocationSet):
                continue
            name = alloc.memorylocations[0].name
            if alloc.kind == "ExternalInput":
                if name != part_name:
                    in_names.append(name)
            elif alloc.kind == "ExternalOutput":
                out_names.append(name)
                out_avals.append(jax.core.ShapedArray(
                    tuple(alloc.tensor_shape), mybir.dt.np(alloc.dtype)))
        self.in_names, self.out_names, self.out_avals = in_names, out_names, out_avals
        n_params = len(in_names)
        self.zero_outs = [np.zeros(a.shape, a.dtype) for a in out_avals]
        all_in_names = tuple(in_names + out_names)
        if part_name is not None:
            all_in_names = all_in_names + (part_name,)

        def _body(*args):
            operands = list(args)
            if part_name is not None:
                operands.append(bass2jax.partition_id_tensor())
            outs = bass2jax._bass_exec_p.bind(
                *operands,
                out_avals=tuple(out_avals),
                in_names=all_in_names,
                out_names=tuple(out_names),
                lowering_input_output_aliases=(),
                sim_require_finite=True,
                sim_require_nnan=True,
                nc=nc,
            )
            return tuple(outs)

        devices = jax.devices()[:n_cores]
        mesh = Mesh(np.asarray(devices), ("core",))
        n_out = len(out_names)
        self._fn = jax.jit(
            shard_map(
                _body, mesh=mesh,
                in_specs=(PartitionSpec("core"),) * (n_params + n_out),
                out_specs=(PartitionSpec("core"),) * n_out,
                check_rep=False,
            ),
            keep_unused=True,
        )

    def __call__(self, in_maps):
        n = self.n_cores
        concat_in = [
            np.concatenate([in_maps[c][name] for c in range(n)], axis=0)
            for name in self.in_names
        ]
        concat_zeros = [
            np.zeros((n * z.shape[0], *z.shape[1:]), z.dtype)
            for z in self.zero_outs
        ]
        out_arrs = self._fn(*concat_in, *concat_zeros)
        return [
            {name: np.asarray(out_arrs[i]).reshape(n, *self.out_avals[i].shape)[c]
             for i, name in enumerate(self.out_names)}
            for c in range(n)
        ]


_RUNNER = {}


def _get_runner():
    if "r" not in _RUNNER:
        _RUNNER["r"] = _PjrtRunner(_get_module(), N_CORES)
    return _RUNNER["r"]


def kernel(**inputs):
    f32 = np.float32
    F_q = np.asarray(inputs["F_q"], f32)
    query = np.asarray(inputs["query"], f32).reshape(-1)
    memory_nodes = np.asarray(inputs["memory_nodes"], f32)
    H = np.asarray(inputs["H"], f32)
    R = np.asarray(inputs["R"], f32)
    a_out = float(np.asarray(inputs["a_out"]).reshape(-1)[0])

    in_maps = _make_in_maps(N_CORES, D, M, L, F_q, query, memory_nodes,
                            H, R, a_out)
    results = _get_runner()(in_maps)
    y = np.concatenate(
        [np.ascontiguousarray(results[c]["y"].T).reshape(-1)
         for c in range(N_CORES)])
    return y.reshape(D, 1).astype(f32)
